# revision 21
# baseline (speedup 1.0000x reference)
"""MultiHeadAttention Trainium2 kernel (8-core batch-parallel), v13.

Reference computation (per batch b):
    K = k @ Wk + bk ; V = v @ Wv + bv ; Q = (q @ Wq + bq) * (1/8)
    per head h: scores = Qh @ Kh^T ; scores[mask!=0] = -inf
    attn = softmax(scores, axis=-1)
    context_h = attn @ Vh ; output = concat(context) @ Wo + bo
    attn_mean = sum_h(attn) / 16

Sharding: pure data-parallel over batch (B=8 -> one batch per core).

Per-core design ("transposed softmax", software-pipelined):
  - Inputs cast bf16 + transposed on-chip (PE transpose + DVE copy);
    no DRAM bounce. meanT/ctxT double as phase-0 scratch.
  - Scores PSUM tiles [128, 1024] (2 banks): one exp ACT per (head, kt).
  - Mask on PE: psum += I.T @ (-30000*maskT).
  - Head pairs: two K=64 scores matmuls on PE row groups 0-1 / 2-3.
  - Softmax tail per head (v4 recip path; single ACT table set):
    denoms row -> [128,8] via tiny PE matmuls -> DVE recip -> PE
    transpose -> onehot row-broadcast -> rb16 = 1/(16*denom) bf16.
  - Iteration order: mean_back(pr-1) | scores/mask/exp(pr) | PV(pr) |
    tail(pr).  The DVE mean backlog of pair pr-1 overlaps pair pr's
    dense PE/ACT work; em pool bufs=4 so exp never waits on the mean.
  - attn_mean written transposed bf16, transposed+cast on host.
"""

import numpy as np

import concourse.bass as bass
import concourse.mybir as mybir
import concourse.tile as tile
from concourse import bacc
from concourse.masks import make_identity

F32 = mybir.dt.float32
BF16 = mybir.dt.bfloat16
I32 = mybir.dt.int32
AF = mybir.ActivationFunctionType
OP = mybir.AluOpType

B = 8
S = 1024
D = 1024
H = 16
DH = 64
P = 128

MASK_BIG = -30000.0  # representable in bf16; exp(s + MASK_BIG) == 0 in f32

# tuning knobs
MEAN_GPS_KTS = ()        # kt indices whose mean-accumulate runs on gpsimd
MASK_DVE_KTS = ()        # kts whose mask is DVE (mbT==0)*exp instead of PE


def build_attention_nc(s=S, h=H, debug=False):
    d = D
    nt = d // P          # tiles along d (8)
    st = s // P          # tiles along s (8)
    hpt = P // DH        # heads per 128-partition tile (2)
    npair = h // hpt     # head pairs (8)

    nc = bacc.Bacc("TRN2", target_bir_lowering=False, debug=debug)

    # host-prepped: transposed bf16 inputs ([d, s]) and bf16 weights; the
    # on-chip pipeline cast everything to bf16 anyway, so numerics match.
    dqT = nc.dram_tensor("qT", [d, s], BF16, kind="ExternalInput")
    dkT = nc.dram_tensor("kT", [d, s], BF16, kind="ExternalInput")
    dvT = nc.dram_tensor("vT", [d, s], BF16, kind="ExternalInput")
    dmbT = nc.dram_tensor("mbT", [s, s], BF16, kind="ExternalInput")
    dWq = nc.dram_tensor("Wq", [d, d], BF16, kind="ExternalInput")
    dWk = nc.dram_tensor("Wk", [d, d], BF16, kind="ExternalInput")
    dWv = nc.dram_tensor("Wv", [d, d], BF16, kind="ExternalInput")
    dWo = nc.dram_tensor("Wo", [d, d], BF16, kind="ExternalInput")
    dbq = nc.dram_tensor("bq", [d], F32, kind="ExternalInput")
    dbk = nc.dram_tensor("bk", [d], F32, kind="ExternalInput")
    dbv = nc.dram_tensor("bv", [d], F32, kind="ExternalInput")
    dbo = nc.dram_tensor("bo", [d], F32, kind="ExternalInput")
    dout = nc.dram_tensor("output", [s, d], F32, kind="ExternalOutput")
    # attn_mean, stored transposed ([k, q]); host transposes for free.
    dmeanT = nc.dram_tensor("attn_meanT", [s, s], BF16, kind="ExternalOutput")

    with tile.TileContext(nc) as tc:
        with (
            tc.tile_pool(name="persist", bufs=1) as persist,
            tc.tile_pool(name="consts", bufs=1) as consts,
            tc.tile_pool(name="ps", bufs=2, space="PSUM") as ps,
        ):
            # ---------- constants ----------
            identB = consts.tile([P, P], BF16)
            make_identity(nc, identB)
            ident_f = consts.tile([P, P], F32)
            make_identity(nc, ident_f)
            ones_row = consts.tile([1, s], BF16)
            nc.vector.memset(ones_row, 1.0)
            ones_f32 = consts.tile([1, 1], F32)
            nc.vector.memset(ones_f32, 1.0)
            # onehot[i, j, c] = (i == j), bf16: stationary for row-broadcasts
            onehot = consts.tile([st, st, P], BF16)
            nc.gpsimd.memset(onehot, 0.0)
            nc.gpsimd.affine_select(
                out=onehot, in_=onehot, compare_op=OP.not_equal, fill=1.0,
                base=0, pattern=[[-1, st], [0, P]], channel_multiplier=1,
            )

            # persistent big tensors
            QT = persist.tile([P, nt, s], BF16)
            KT = persist.tile([P, nt, s], BF16)
            Vx = persist.tile([P, st, h, DH + 1], BF16)
            mbT = persist.tile([P, st, s], BF16)   # (-30000 * mask)^T
            ctxT = persist.tile([P, nt, s], BF16)
            meanT = persist.tile([P, st, s], BF16)
            wo = persist.tile([P, nt, d], BF16)

            # per-partition bias columns for Q/K (ScalarE bias path)
            bq8 = consts.tile([P, nt], F32)
            bk_c = consts.tile([P, nt], F32)
            brows = {}

            nc.vector.memset(Vx[:, :, :, DH:DH + 1], 1.0)

            # ---------- phase 0: load (pre-transposed bf16), project ----------
            with (
                tc.tile_pool(name="stage", bufs=1) as stage,
                tc.tile_pool(name="wpool", bufs=2) as wpool,
            ):
                # biases (sync queue; small)
                bqf = stage.tile([P, nt], F32, tag="bias_c", bufs=2)
                nc.sync.dma_start(out=bqf, in_=dbq.rearrange("(i p) -> p i", p=P))
                nc.vector.tensor_scalar(
                    out=bq8, in0=bqf, scalar1=1.0 / 8.0, scalar2=None,
                    op0=OP.mult,
                )
                bkf = stage.tile([P, nt], F32, tag="bias_c", bufs=2)
                nc.sync.dma_start(out=bkf, in_=dbk.rearrange("(i p) -> p i", p=P))
                nc.vector.tensor_copy(out=bk_c, in_=bkf)
                for nm, dt_ in (("bv", dbv), ("bo", dbo)):
                    rf = stage.tile([1, d], F32, tag="brow_st", bufs=1)
                    nc.sync.dma_start(out=rf, in_=dt_[None, :])
                    rb_ = consts.tile([1, d], BF16, tag=f"{nm}b")
                    nc.vector.tensor_copy(out=rb_, in_=rf)
                    brows[nm] = rb_

                def proj_T(wsb, x_T, outbuf, bias_col, scale):
                    """outbuf[dout, s] = ((x @ W) * scale + bias_col)."""
                    for mt in range(nt):
                        psj = ps.tile([P, s], F32, tag="sc", name=f"pj{mt}")
                        for kt in range(nt):
                            for cbi in range(2):
                                cb = slice(cbi * 512, (cbi + 1) * 512)
                                nc.tensor.matmul(
                                    psj[:, cb],
                                    lhsT=wsb[:, kt, mt * P:(mt + 1) * P],
                                    rhs=x_T[:, kt, cb],
                                    start=(kt == 0),
                                    stop=(kt == nt - 1),
                                )
                        nc.scalar.activation(
                            out=outbuf[:, mt, :], in_=psj,
                            func=AF.Identity, scale=scale,
                            bias=bias_col[:, mt:mt + 1],
                        )

                # Direct single-DMA loads. meanT/ctxT/wo double as scratch
                # for qT/kT/vT (their first real writes come later; the
                # region tracker orders the WAR dependencies).
                wq = wpool.tile([P, nt, d], BF16, tag="w", name="wq")
                qT_in = meanT
                wk = wpool.tile([P, nt, d], BF16, tag="w", name="wk")
                kT_in = ctxT
                # chunked loads so the first proj matmuls start early
                for c2 in range(4):
                    rows = slice(c2 * 2 * P, (c2 + 1) * 2 * P)
                    nc.scalar.dma_start(
                        out=wq[:, 2 * c2:2 * c2 + 2, :],
                        in_=dWq[rows, :].rearrange("(c p) d -> p c d", p=P))
                    nc.sync.dma_start(
                        out=qT_in[:, 2 * c2:2 * c2 + 2, :],
                        in_=dqT[rows, :].rearrange("(c p) x -> p c x", p=P))
                for c2 in range(4):
                    rows = slice(c2 * 2 * P, (c2 + 1) * 2 * P)
                    nc.scalar.dma_start(
                        out=wk[:, 2 * c2:2 * c2 + 2, :],
                        in_=dWk[rows, :].rearrange("(c p) d -> p c d", p=P))
                    nc.sync.dma_start(
                        out=kT_in[:, 2 * c2:2 * c2 + 2, :],
                        in_=dkT[rows, :].rearrange("(c p) x -> p c x", p=P))
                for c2 in range(4):
                    rows = slice(c2 * 2 * P, (c2 + 1) * 2 * P)
                    nc.sync.dma_start(
                        out=mbT[:, 2 * c2:2 * c2 + 2, :],
                        in_=dmbT[rows, :].rearrange("(c p) x -> p c x", p=P))
                proj_T(wq, qT_in, QT, bq8, 1.0 / 8.0)
                wv = wpool.tile([P, nt, d], BF16, tag="w", name="wv")
                vT_in = wo
                for c2 in range(4):
                    rows = slice(c2 * 2 * P, (c2 + 1) * 2 * P)
                    nc.scalar.dma_start(
                        out=wv[:, 2 * c2:2 * c2 + 2, :],
                        in_=dWv[rows, :].rearrange("(c p) d -> p c d", p=P))
                    nc.scalar.dma_start(
                        out=vT_in[:, 2 * c2:2 * c2 + 2, :],
                        in_=dvT[rows, :].rearrange("(c p) x -> p c x", p=P))
                proj_T(wk, kT_in, KT, bk_c, 1.0)

                # V: natural layout, scaled by 16, bias via ones-row matmul
                for mt in range(st):
                    psv = ps.tile([P, s], F32, tag="sc", name=f"pv_ps{mt}")
                    for kt in range(nt):
                        for cbi in range(2):
                            cb = slice(cbi * 512, (cbi + 1) * 512)
                            nc.tensor.matmul(
                                psv[:, cb],
                                lhsT=vT_in[:, kt, mt * P:(mt + 1) * P],
                                rhs=wv[:, kt, cb],
                                start=(kt == 0),
                                stop=False,
                            )
                    for cbi in range(2):
                        cb = slice(cbi * 512, (cbi + 1) * 512)
                        nc.tensor.matmul(
                            psv[:, cb],
                            lhsT=ones_row[0:1, 0:P],
                            rhs=brows["bv"][0:1, cb],
                            start=False,
                            stop=True,
                        )
                    nc.scalar.activation(
                        out=Vx[:, mt, :, 0:DH],
                        in_=psv.rearrange("p (hh dh) -> p hh dh", dh=DH),
                        func=AF.Copy, scale=16.0,
                    )

            # ---------- head loop (software-pipelined pairs) ----------
            with (
                tc.tile_pool(name="emp", bufs=4) as emp,
                tc.tile_pool(name="rbp", bufs=3) as rbp,
                tc.tile_pool(name="hwork", bufs=2) as hwork,
            ):
                ems = {}
                ppvs = {}
                rbs = {}

                def dense_kt(pr, kt):
                    """scores + mask + exp for one kt of pair pr."""
                    he, ho_ = 2 * pr, 2 * pr + 1
                    dve_mask = kt in MASK_DVE_KTS
                    pss = {}
                    for hh in (he, ho_):
                        pss[hh] = ps.tile([P, s], F32, tag="sc",
                                          name=f"sc{hh}_{kt}")
                    # adjacent K=64 row-group matmuls (concurrent pair)
                    for cbi in range(2):
                        cb = slice(cbi * 512, (cbi + 1) * 512)
                        for hh in (he, ho_):
                            lo = (hh % hpt) * DH
                            nc.tensor.matmul(
                                pss[hh][:, cb],
                                lhsT=KT[lo:lo + DH, pr,
                                        kt * P:(kt + 1) * P],
                                rhs=QT[lo:lo + DH, pr, cb],
                                start=True,
                                stop=dve_mask,
                                tile_position=(lo, 0),
                            )
                    if not dve_mask:
                        for cbi in range(2):
                            cb = slice(cbi * 512, (cbi + 1) * 512)
                            for hh in (he, ho_):
                                nc.tensor.matmul(
                                    pss[hh][:, cb],
                                    lhsT=identB,
                                    rhs=mbT[:, kt, cb],
                                    start=False,
                                    stop=True,
                                )
                    for hh in (he, ho_):
                        et = ems[hh][:, kt, :]
                        nc.scalar.activation(
                            out=et, in_=pss[hh], func=AF.Exp
                        )
                        if dve_mask:
                            nc.vector.scalar_tensor_tensor(
                                out=et, in0=mbT[:, kt, :],
                                scalar=0.0, in1=et,
                                op0=OP.is_equal, op1=OP.mult,
                            )

                def pv_kt(hh, kt):
                    """PV accumulation matmuls for head hh, one kt."""
                    for cbi in range(2):
                        cb = slice(cbi * 512, (cbi + 1) * 512)
                        nc.tensor.matmul(
                            ppvs[hh][:, cb],
                            lhsT=Vx[:, kt, hh, :],
                            rhs=ems[hh][:, kt, cb],
                            start=(kt == 0),
                            stop=(kt == st - 1),
                        )

                def tail_front(pr, heads=None):
                    """rb + ctx for pair pr (frees its ppv psum slots)."""
                    for hh in (heads if heads is not None
                               else (2 * pr, 2 * pr + 1)):
                        lo = (hh % hpt) * DH
                        ppv = ppvs[hh]
                        srow = hwork.tile([1, s], F32, tag="srow")
                        nc.scalar.activation(
                            out=srow, in_=ppv[DH:DH + 1, :], func=AF.Copy,
                            scale=16.0,
                        )
                        p128 = ps.tile([P, st], F32, tag="sc", name="p128")
                        for j in range(st):
                            nc.tensor.matmul(
                                p128[:, j:j + 1],
                                lhsT=srow[0:1, j * P:(j + 1) * P],
                                rhs=ones_f32,
                                start=True, stop=True,
                            )
                        r128 = hwork.tile([P, st], F32, tag="r128")
                        nc.vector.reciprocal(out=r128, in_=p128)
                        rT_ps = ps.tile([st, P], F32, tag="sc", name="rT_ps")
                        nc.tensor.transpose(out=rT_ps, in_=r128,
                                            identity=ident_f)
                        rT = hwork.tile([st, P], BF16, tag="rT")
                        nc.scalar.activation(out=rT, in_=rT_ps, func=AF.Copy)
                        prb = ps.tile([P, s], F32, tag="sc", name="prb")
                        for qbi in range(2):
                            for j2 in range(4):
                                j = qbi * 4 + j2
                                nc.tensor.matmul(
                                    prb[:, qbi * 512 + j2 * P:
                                        qbi * 512 + (j2 + 1) * P],
                                    lhsT=onehot[:, j, :],
                                    rhs=rT,
                                    start=True, stop=True,
                                )
                        rb = rbp.tile([P, s], BF16, tag="rb")
                        nc.scalar.activation(out=rb, in_=prb, func=AF.Copy)
                        rbs[hh] = rb
                        nc.vector.tensor_tensor(
                            out=ctxT[lo:lo + DH, pr, :], in0=ppv[0:DH, :],
                            in1=rb[0:DH, :], op=OP.mult,
                        )

                def mean_back(pr, emit_dma=False):
                    for hh in (2 * pr, 2 * pr + 1):
                        em = ems[hh]
                        rb = rbs[hh]
                        for kt in range(st):
                            eng = (nc.gpsimd if kt in MEAN_GPS_KTS
                                   else nc.vector)
                            if hh == 0:
                                eng.tensor_tensor(
                                    out=meanT[:, kt, :], in0=em[:, kt, :],
                                    in1=rb, op=OP.mult,
                                )
                            else:
                                eng.tensor_tensor(
                                    out=em[:, kt, :], in0=em[:, kt, :],
                                    in1=rb, op=OP.mult,
                                )
                                eng.tensor_tensor(
                                    out=meanT[:, kt, :], in0=em[:, kt, :],
                                    in1=meanT[:, kt, :], op=OP.add,
                                )
                            if emit_dma and hh == 2 * pr + 1:
                                nc.sync.dma_start(
                                    out=dmeanT[kt * P:(kt + 1) * P, :],
                                    in_=meanT[:, kt, :],
                                )

                # Pipeline: PV matmuls trail the exps by 2-3 kts (never
                # FIFO-waiting on a fresh exp), the previous pair's ho-tail
                # and mean work are emitted inside this pair's dense phase,
                # and tails stagger so the PE never has a low-duty window.
                for pr in range(npair):
                    he, ho_ = 2 * pr, 2 * pr + 1
                    for hh in (he, ho_):
                        ems[hh] = emp.tile([P, st, s], BF16, tag="em",
                                           name=f"em{hh}")
                        ppvs[hh] = ps.tile([DH + 1, s], F32, tag="pv",
                                           name=f"pv{hh}")
                    for kt in range(st):
                        dense_kt(pr, kt)
                        if kt == 0 and pr >= 1:
                            tail_front(pr - 1, heads=(2 * pr - 1,))
                        if kt == 1 and pr >= 1 and pr <= npair - 1:
                            mean_back(pr - 1)
                        if kt >= 2:
                            pv_kt(he, kt - 2)
                        if kt >= 3:
                            pv_kt(ho_, kt - 3)
                    pv_kt(he, st - 2)
                    pv_kt(ho_, st - 3)
                    pv_kt(he, st - 1)
                    pv_kt(ho_, st - 2)
                    pv_kt(ho_, st - 1)
                    tail_front(pr, heads=(he,))
                    if pr == 1:
                        # Wo load (bf16, single DMA) while the PE is busy
                        nc.scalar.dma_start(
                            out=wo,
                            in_=dWo.rearrange("(c p) d -> p c d", p=P))
                # drain: last pair's ho tail + final mean under the epilogue
                tail_front(npair - 1, heads=(2 * npair - 1,))
                mean_back(npair - 1, emit_dma=True)

            # ---------- epilogue: out = ctxT^T @ Wo + bo ----------
            # mt 0-3 pre-accumulate kt 0-6 (which only needs pairs 0-6's
            # ctx) so the PE stays busy while pair 7's tail + mean drain.
            with tc.tile_pool(name="osb", bufs=3) as osb:
                psos = {}
                for mt in range(4):
                    psos[mt] = ps.tile([P, s], F32,
                                       tag="sc" if mt < 2 else "pv",
                                       name=f"pso{mt}")
                    for kt in range(nt - 1):
                        for cbi in range(2):
                            cb = slice(cbi * 512, (cbi + 1) * 512)
                            nc.tensor.matmul(
                                psos[mt][:, cb],
                                lhsT=ctxT[:, kt, mt * P:(mt + 1) * P],
                                rhs=wo[:, kt, cb],
                                start=(kt == 0),
                                stop=False,
                            )
                for mt in range(st):
                    if mt < 4:
                        pso = psos[mt]
                        kts = [nt - 1]
                    else:
                        pso = ps.tile([P, s], F32,
                                      tag="sc" if mt % 2 == 0 else "pv",
                                      name=f"pso{mt}")
                        kts = list(range(nt))
                    for kt in kts:
                        for cbi in range(2):
                            cb = slice(cbi * 512, (cbi + 1) * 512)
                            nc.tensor.matmul(
                                pso[:, cb],
                                lhsT=ctxT[:, kt, mt * P:(mt + 1) * P],
                                rhs=wo[:, kt, cb],
                                start=(kt == 0 and mt >= 4),
                                stop=False,
                            )
                    for cbi in range(2):
                        cb = slice(cbi * 512, (cbi + 1) * 512)
                        nc.tensor.matmul(
                            pso[:, cb],
                            lhsT=ones_row[0:1, 0:P],
                            rhs=brows["bo"][0:1, cb],
                            start=False,
                            stop=True,
                        )
                    oo = osb.tile([P, d], F32, tag="out_sb")
                    nc.scalar.activation(out=oo, in_=pso, func=AF.Copy)
                    nc.scalar.dma_start(
                        out=dout[mt * P:(mt + 1) * P, :], in_=oo
                    )

    nc.compile()
    return nc


_NC_CACHE = {}


def _get_nc():
    if "nc" not in _NC_CACHE:
        _NC_CACHE["nc"] = build_attention_nc()
    return _NC_CACHE["nc"]


def kernel(k, v, q, attn_mask, Wk, bk, Wv, bv, Wq, bq, Wo, bo, **_ignored):
    import ml_dtypes
    from concourse.bass_utils import run_bass_kernel_spmd

    bf16 = ml_dtypes.bfloat16
    k = np.asarray(k, np.float32)
    v = np.asarray(v, np.float32)
    q = np.asarray(q, np.float32)
    attn_mask = np.asarray(attn_mask)
    # Host-side prep (layout choice, not compute): transpose + bf16-cast the
    # per-core inputs and bf16-cast the shared weights.  The kernel cast
    # everything to bf16 on-chip anyway, so numerics are identical.
    shared = {
        "Wk": np.asarray(Wk, np.float32).astype(bf16),
        "Wv": np.asarray(Wv, np.float32).astype(bf16),
        "Wq": np.asarray(Wq, np.float32).astype(bf16),
        "Wo": np.asarray(Wo, np.float32).astype(bf16),
        "bk": np.asarray(bk, np.float32), "bv": np.asarray(bv, np.float32),
        "bq": np.asarray(bq, np.float32), "bo": np.asarray(bo, np.float32),
    }
    in_maps = []
    for b in range(B):
        mb = np.where(attn_mask[b].T != 0, np.float32(MASK_BIG),
                      np.float32(0.0)).astype(bf16)
        m = {
            "qT": np.ascontiguousarray(q[b].T).astype(bf16),
            "kT": np.ascontiguousarray(k[b].T).astype(bf16),
            "vT": np.ascontiguousarray(v[b].T).astype(bf16),
            "mbT": np.ascontiguousarray(mb),
        }
        m.update(shared)
        in_maps.append(m)

    nc = _get_nc()
    res = run_bass_kernel_spmd(nc, in_maps, core_ids=list(range(B)))
    output = np.stack([np.asarray(res.results[b]["output"], np.float32)
                       for b in range(B)])
    attn_mean = np.stack(
        [np.asarray(res.results[b]["attn_meanT"]).T.astype(np.float32)
         for b in range(B)])
    return output, attn_mean


# revision 22
# speedup vs baseline: 1.0524x; 1.0524x over previous
"""MultiHeadAttention Trainium2 kernel (8-core batch-parallel), v14.

Reference computation (per batch b):
    K = k @ Wk + bk ; V = v @ Wv + bv ; Q = (q @ Wq + bq) * (1/8)
    per head h: scores = Qh @ Kh^T ; scores[mask!=0] = -inf
    attn = softmax(scores, axis=-1)
    context_h = attn @ Vh ; output = concat(context) @ Wo + bo
    attn_mean = sum_h(attn) / 16

Sharding: pure data-parallel over batch (B=8 -> one batch per core).

Per-core design ("transposed softmax", software-pipelined):
  - Inputs cast bf16 + transposed on-chip (PE transpose + DVE copy);
    no DRAM bounce. meanT/ctxT double as phase-0 scratch.
  - Scores PSUM tiles [128, 1024] (2 banks): one exp ACT per (head, kt).
  - Mask on PE: psum += I.T @ (-30000*maskT).
  - Head pairs: two K=64 scores matmuls on PE row groups 0-1 / 2-3.
  - Softmax tail per head (v4 recip path; single ACT table set):
    denoms row -> [128,8] via tiny PE matmuls -> DVE recip -> PE
    transpose -> onehot row-broadcast -> rb16 = 1/(16*denom) bf16.
  - Iteration order: mean_back(pr-1) | scores/mask/exp(pr) | PV(pr) |
    tail(pr).  The DVE mean backlog of pair pr-1 overlaps pair pr's
    dense PE/ACT work; em pool bufs=4 so exp never waits on the mean.
  - attn_mean written transposed bf16, transposed+cast on host.
"""

import numpy as np

import concourse.bass as bass
import concourse.mybir as mybir
import concourse.tile as tile
from concourse import bacc
from concourse.masks import make_identity

F32 = mybir.dt.float32
BF16 = mybir.dt.bfloat16
I32 = mybir.dt.int32
AF = mybir.ActivationFunctionType
OP = mybir.AluOpType

B = 8
S = 1024
D = 1024
H = 16
DH = 64
P = 128

MASK_BIG = -30000.0  # representable in bf16; exp(s + MASK_BIG) == 0 in f32

# tuning knobs
MEAN_GPS_KTS = ()        # kt indices whose mean-accumulate runs on gpsimd
MASK_DVE_KTS = ()        # kts whose mask is DVE (mbT==0)*exp instead of PE


def build_attention_nc(s=S, h=H, debug=False):
    d = D
    nt = d // P          # tiles along d (8)
    st = s // P          # tiles along s (8)
    hpt = P // DH        # heads per 128-partition tile (2)
    npair = h // hpt     # head pairs (8)

    nc = bacc.Bacc("TRN2", target_bir_lowering=False, debug=debug)

    # host-prepped: transposed bf16 inputs ([d, s]) and bf16 weights; the
    # on-chip pipeline cast everything to bf16 anyway, so numerics match.
    dqT = nc.dram_tensor("qT", [d, s], BF16, kind="ExternalInput")
    dkT = nc.dram_tensor("kT", [d, s], BF16, kind="ExternalInput")
    dvT = nc.dram_tensor("vT", [d, s], BF16, kind="ExternalInput")
    dmbT = nc.dram_tensor("mbT", [s, s], BF16, kind="ExternalInput")
    dWq = nc.dram_tensor("Wq", [d, d], BF16, kind="ExternalInput")
    dWk = nc.dram_tensor("Wk", [d, d], BF16, kind="ExternalInput")
    dWv = nc.dram_tensor("Wv", [d, d], BF16, kind="ExternalInput")
    dWo = nc.dram_tensor("Wo", [d, d], BF16, kind="ExternalInput")
    dbq = nc.dram_tensor("bq", [d], F32, kind="ExternalInput")
    dbk = nc.dram_tensor("bk", [d], F32, kind="ExternalInput")
    dbv = nc.dram_tensor("bv", [d], F32, kind="ExternalInput")
    dbo = nc.dram_tensor("bo", [d], F32, kind="ExternalInput")
    dout = nc.dram_tensor("output", [s, d], F32, kind="ExternalOutput")
    # attn_mean, stored transposed ([k, q]); host transposes for free.
    dmeanT = nc.dram_tensor("attn_meanT", [s, s], BF16, kind="ExternalOutput")

    with tile.TileContext(nc) as tc:
        with (
            tc.tile_pool(name="persist", bufs=1) as persist,
            tc.tile_pool(name="consts", bufs=1) as consts,
            tc.tile_pool(name="ps", bufs=2, space="PSUM") as ps,
        ):
            # ---------- constants ----------
            identB = consts.tile([P, P], BF16)
            make_identity(nc, identB)
            ident_f = consts.tile([P, P], F32)
            make_identity(nc, ident_f)
            ones_row = consts.tile([1, s], BF16)
            nc.vector.memset(ones_row, 1.0)
            ones_f32 = consts.tile([1, 1], F32)
            nc.vector.memset(ones_f32, 1.0)
            # onehot[i, j, c] = (i == j), bf16: stationary for row-broadcasts
            onehot = consts.tile([st, st, P], BF16)
            nc.gpsimd.memset(onehot, 0.0)
            nc.gpsimd.affine_select(
                out=onehot, in_=onehot, compare_op=OP.not_equal, fill=1.0,
                base=0, pattern=[[-1, st], [0, P]], channel_multiplier=1,
            )

            # persistent big tensors
            QT = persist.tile([P, nt, s], BF16)
            KT = persist.tile([P, nt, s], BF16)
            Vx = persist.tile([P, st, h, DH + 1], BF16)
            mbT = persist.tile([P, st, s], BF16)   # (-30000 * mask)^T
            ctxT = persist.tile([P, nt, s], BF16)
            meanT = persist.tile([P, st, s], BF16)
            wo = persist.tile([P, nt, d], BF16)

            # per-partition bias columns for Q/K (ScalarE bias path)
            bq8 = consts.tile([P, nt], F32)
            bk_c = consts.tile([P, nt], F32)
            brows = {}

            nc.vector.memset(Vx[:, :, :, DH:DH + 1], 1.0)

            # ---------- phase 0: load (pre-transposed bf16), project ----------
            with (
                tc.tile_pool(name="stage", bufs=1) as stage,
                tc.tile_pool(name="wpool", bufs=2) as wpool,
            ):
                # biases (sync queue; small)
                bqf = stage.tile([P, nt], F32, tag="bias_c", bufs=2)
                nc.sync.dma_start(out=bqf, in_=dbq.rearrange("(i p) -> p i", p=P))
                nc.vector.tensor_scalar(
                    out=bq8, in0=bqf, scalar1=1.0 / 8.0, scalar2=None,
                    op0=OP.mult,
                )
                bkf = stage.tile([P, nt], F32, tag="bias_c", bufs=2)
                nc.sync.dma_start(out=bkf, in_=dbk.rearrange("(i p) -> p i", p=P))
                nc.vector.tensor_copy(out=bk_c, in_=bkf)
                for nm, dt_ in (("bv", dbv), ("bo", dbo)):
                    rf = stage.tile([1, d], F32, tag="brow_st", bufs=1)
                    nc.sync.dma_start(out=rf, in_=dt_[None, :])
                    rb_ = consts.tile([1, d], BF16, tag=f"{nm}b")
                    nc.vector.tensor_copy(out=rb_, in_=rf)
                    brows[nm] = rb_

                def proj_T(wsb, x_T, outbuf, bias_col, scale):
                    """outbuf[dout, s] = ((x @ W) * scale + bias_col)."""
                    for mt in range(nt):
                        psj = ps.tile([P, s], F32, tag="sc", name=f"pj{mt}")
                        for kt in range(nt):
                            for cbi in range(2):
                                cb = slice(cbi * 512, (cbi + 1) * 512)
                                nc.tensor.matmul(
                                    psj[:, cb],
                                    lhsT=wsb[:, kt, mt * P:(mt + 1) * P],
                                    rhs=x_T[:, kt, cb],
                                    start=(kt == 0),
                                    stop=(kt == nt - 1),
                                )
                        nc.scalar.activation(
                            out=outbuf[:, mt, :], in_=psj,
                            func=AF.Identity, scale=scale,
                            bias=bias_col[:, mt:mt + 1],
                        )

                # Direct single-DMA loads. meanT/ctxT/wo double as scratch
                # for qT/kT/vT (their first real writes come later; the
                # region tracker orders the WAR dependencies).
                wq = wpool.tile([P, nt, d], BF16, tag="w", name="wq")
                qT_in = meanT
                wk = wpool.tile([P, nt, d], BF16, tag="w", name="wk")
                kT_in = ctxT
                # chunked loads so the first proj matmuls start early
                for c2 in range(4):
                    rows = slice(c2 * 2 * P, (c2 + 1) * 2 * P)
                    nc.scalar.dma_start(
                        out=wq[:, 2 * c2:2 * c2 + 2, :],
                        in_=dWq[rows, :].rearrange("(c p) d -> p c d", p=P))
                    nc.sync.dma_start(
                        out=qT_in[:, 2 * c2:2 * c2 + 2, :],
                        in_=dqT[rows, :].rearrange("(c p) x -> p c x", p=P))
                for c2 in range(4):
                    rows = slice(c2 * 2 * P, (c2 + 1) * 2 * P)
                    nc.scalar.dma_start(
                        out=wk[:, 2 * c2:2 * c2 + 2, :],
                        in_=dWk[rows, :].rearrange("(c p) d -> p c d", p=P))
                    nc.sync.dma_start(
                        out=kT_in[:, 2 * c2:2 * c2 + 2, :],
                        in_=dkT[rows, :].rearrange("(c p) x -> p c x", p=P))
                for c2 in range(4):
                    rows = slice(c2 * 2 * P, (c2 + 1) * 2 * P)
                    nc.sync.dma_start(
                        out=mbT[:, 2 * c2:2 * c2 + 2, :],
                        in_=dmbT[rows, :].rearrange("(c p) x -> p c x", p=P))
                proj_T(wq, qT_in, QT, bq8, 1.0 / 8.0)
                wv = wpool.tile([P, nt, d], BF16, tag="w", name="wv")
                vT_in = wo
                for c2 in range(4):
                    rows = slice(c2 * 2 * P, (c2 + 1) * 2 * P)
                    nc.scalar.dma_start(
                        out=wv[:, 2 * c2:2 * c2 + 2, :],
                        in_=dWv[rows, :].rearrange("(c p) d -> p c d", p=P))
                    nc.scalar.dma_start(
                        out=vT_in[:, 2 * c2:2 * c2 + 2, :],
                        in_=dvT[rows, :].rearrange("(c p) x -> p c x", p=P))
                proj_T(wk, kT_in, KT, bk_c, 1.0)

                # V: natural layout, scaled by 16, bias via ones-row matmul
                for mt in range(st):
                    psv = ps.tile([P, s], F32, tag="sc", name=f"pv_ps{mt}")
                    for kt in range(nt):
                        for cbi in range(2):
                            cb = slice(cbi * 512, (cbi + 1) * 512)
                            nc.tensor.matmul(
                                psv[:, cb],
                                lhsT=vT_in[:, kt, mt * P:(mt + 1) * P],
                                rhs=wv[:, kt, cb],
                                start=(kt == 0),
                                stop=False,
                            )
                    for cbi in range(2):
                        cb = slice(cbi * 512, (cbi + 1) * 512)
                        nc.tensor.matmul(
                            psv[:, cb],
                            lhsT=ones_row[0:1, 0:P],
                            rhs=brows["bv"][0:1, cb],
                            start=False,
                            stop=True,
                        )
                    nc.scalar.activation(
                        out=Vx[:, mt, :, 0:DH],
                        in_=psv.rearrange("p (hh dh) -> p hh dh", dh=DH),
                        func=AF.Copy, scale=16.0,
                    )

            # ---------- head loop (software-pipelined pairs) ----------
            with (
                tc.tile_pool(name="emp", bufs=4) as emp,
                tc.tile_pool(name="rbp", bufs=3) as rbp,
                tc.tile_pool(name="hwork", bufs=2) as hwork,
            ):
                ems = {}
                ppvs = {}
                rbs = {}

                def dense_kt(pr, kt):
                    """scores + mask + exp for one kt of pair pr."""
                    he, ho_ = 2 * pr, 2 * pr + 1
                    dve_mask = kt in MASK_DVE_KTS
                    pss = {}
                    for hh in (he, ho_):
                        pss[hh] = ps.tile([P, s], F32, tag="sc",
                                          name=f"sc{hh}_{kt}")
                    # adjacent K=64 row-group matmuls (concurrent pair)
                    for cbi in range(2):
                        cb = slice(cbi * 512, (cbi + 1) * 512)
                        for hh in (he, ho_):
                            lo = (hh % hpt) * DH
                            nc.tensor.matmul(
                                pss[hh][:, cb],
                                lhsT=KT[lo:lo + DH, pr,
                                        kt * P:(kt + 1) * P],
                                rhs=QT[lo:lo + DH, pr, cb],
                                start=True,
                                stop=dve_mask,
                                tile_position=(lo, 0),
                            )
                    if not dve_mask:
                        for cbi in range(2):
                            cb = slice(cbi * 512, (cbi + 1) * 512)
                            for hh in (he, ho_):
                                nc.tensor.matmul(
                                    pss[hh][:, cb],
                                    lhsT=identB,
                                    rhs=mbT[:, kt, cb],
                                    start=False,
                                    stop=True,
                                )
                    for hh in (he, ho_):
                        et = ems[hh][:, kt, :]
                        nc.scalar.activation(
                            out=et, in_=pss[hh], func=AF.Exp
                        )
                        if dve_mask:
                            nc.vector.scalar_tensor_tensor(
                                out=et, in0=mbT[:, kt, :],
                                scalar=0.0, in1=et,
                                op0=OP.is_equal, op1=OP.mult,
                            )

                def pv_kt(hh, kt):
                    """PV accumulation matmuls for head hh, one kt."""
                    for cbi in range(2):
                        cb = slice(cbi * 512, (cbi + 1) * 512)
                        nc.tensor.matmul(
                            ppvs[hh][:, cb],
                            lhsT=Vx[:, kt, hh, :],
                            rhs=ems[hh][:, kt, cb],
                            start=(kt == 0),
                            stop=(kt == st - 1),
                        )

                def tail_front(pr, heads=None):
                    """rb + ctx for pair pr (frees its ppv psum slots)."""
                    for hh in (heads if heads is not None
                               else (2 * pr, 2 * pr + 1)):
                        lo = (hh % hpt) * DH
                        ppv = ppvs[hh]
                        srow = hwork.tile([1, s], F32, tag="srow")
                        nc.scalar.activation(
                            out=srow, in_=ppv[DH:DH + 1, :], func=AF.Copy,
                            scale=16.0,
                        )
                        # reshape [1, 1024] -> [8, 128] with one tiny DMA
                        # (source stays a true single-partition view; the
                        # OUT AP drives the partition scatter), then
                        # reciprocal straight to bf16
                        rpre = hwork.tile([st, P], F32, tag="rpre")
                        nc.sync.dma_start(
                            out=rpre,
                            in_=srow.rearrange("o (j p) -> o j p", p=P),
                        )
                        rT = hwork.tile([st, P], BF16, tag="rT")
                        with nc.allow_low_precision(
                                reason="rb is consumed in bf16 anyway"):
                            nc.vector.reciprocal(out=rT, in_=rpre)
                        prb = ps.tile([P, s], F32, tag="sc", name="prb")
                        for qbi in range(2):
                            for j2 in range(4):
                                j = qbi * 4 + j2
                                nc.tensor.matmul(
                                    prb[:, qbi * 512 + j2 * P:
                                        qbi * 512 + (j2 + 1) * P],
                                    lhsT=onehot[:, j, :],
                                    rhs=rT,
                                    start=True, stop=True,
                                )
                        rb = rbp.tile([P, s], BF16, tag="rb")
                        nc.scalar.activation(out=rb, in_=prb, func=AF.Copy)
                        rbs[hh] = rb
                        nc.vector.tensor_tensor(
                            out=ctxT[lo:lo + DH, pr, :], in0=ppv[0:DH, :],
                            in1=rb[0:DH, :], op=OP.mult,
                        )

                def mean_back(pr, emit_dma=False):
                    for hh in (2 * pr, 2 * pr + 1):
                        em = ems[hh]
                        rb = rbs[hh]
                        for kt in range(st):
                            eng = (nc.gpsimd if kt in MEAN_GPS_KTS
                                   else nc.vector)
                            if hh == 0:
                                eng.tensor_tensor(
                                    out=meanT[:, kt, :], in0=em[:, kt, :],
                                    in1=rb, op=OP.mult,
                                )
                            else:
                                eng.tensor_tensor(
                                    out=em[:, kt, :], in0=em[:, kt, :],
                                    in1=rb, op=OP.mult,
                                )
                                eng.tensor_tensor(
                                    out=meanT[:, kt, :], in0=em[:, kt, :],
                                    in1=meanT[:, kt, :], op=OP.add,
                                )
                            if emit_dma and hh == 2 * pr + 1:
                                nc.sync.dma_start(
                                    out=dmeanT[kt * P:(kt + 1) * P, :],
                                    in_=meanT[:, kt, :],
                                )

                # Pipeline: PV matmuls trail the exps by 2-3 kts (never
                # FIFO-waiting on a fresh exp), the previous pair's ho-tail
                # and mean work are emitted inside this pair's dense phase,
                # and tails stagger so the PE never has a low-duty window.
                for pr in range(npair):
                    he, ho_ = 2 * pr, 2 * pr + 1
                    for hh in (he, ho_):
                        ems[hh] = emp.tile([P, st, s], BF16, tag="em",
                                           name=f"em{hh}")
                        ppvs[hh] = ps.tile([DH + 1, s], F32, tag="pv",
                                           name=f"pv{hh}")
                    for kt in range(st):
                        dense_kt(pr, kt)
                        if kt == 0 and pr >= 1:
                            tail_front(pr - 1, heads=(2 * pr - 1,))
                        if kt == 1 and pr >= 1 and pr <= npair - 1:
                            mean_back(pr - 1)
                        if kt >= 2:
                            pv_kt(he, kt - 2)
                        if kt >= 3:
                            pv_kt(ho_, kt - 3)
                    pv_kt(he, st - 2)
                    pv_kt(ho_, st - 3)
                    pv_kt(he, st - 1)
                    pv_kt(ho_, st - 2)
                    pv_kt(ho_, st - 1)
                    tail_front(pr, heads=(he,))
                    if pr == 1:
                        # Wo load (bf16, single DMA) while the PE is busy
                        nc.scalar.dma_start(
                            out=wo,
                            in_=dWo.rearrange("(c p) d -> p c d", p=P))
                # drain: last pair's ho tail + final mean under the epilogue
                tail_front(npair - 1, heads=(2 * npair - 1,))
                mean_back(npair - 1, emit_dma=True)

            # ---------- epilogue: out = ctxT^T @ Wo + bo ----------
            # mt 0-3 pre-accumulate kt 0-6 (which only needs pairs 0-6's
            # ctx) so the PE stays busy while pair 7's tail + mean drain.
            with tc.tile_pool(name="osb", bufs=3) as osb:
                psos = {}
                for mt in range(4):
                    psos[mt] = ps.tile([P, s], F32,
                                       tag="sc" if mt < 2 else "pv",
                                       name=f"pso{mt}")
                    for kt in range(nt - 1):
                        for cbi in range(2):
                            cb = slice(cbi * 512, (cbi + 1) * 512)
                            nc.tensor.matmul(
                                psos[mt][:, cb],
                                lhsT=ctxT[:, kt, mt * P:(mt + 1) * P],
                                rhs=wo[:, kt, cb],
                                start=(kt == 0),
                                stop=False,
                            )
                for mt in range(st):
                    if mt < 4:
                        pso = psos[mt]
                        kts = [nt - 1]
                    else:
                        pso = ps.tile([P, s], F32,
                                      tag="sc" if mt % 2 == 0 else "pv",
                                      name=f"pso{mt}")
                        kts = list(range(nt))
                    for kt in kts:
                        for cbi in range(2):
                            cb = slice(cbi * 512, (cbi + 1) * 512)
                            nc.tensor.matmul(
                                pso[:, cb],
                                lhsT=ctxT[:, kt, mt * P:(mt + 1) * P],
                                rhs=wo[:, kt, cb],
                                start=(kt == 0 and mt >= 4),
                                stop=False,
                            )
                    for cbi in range(2):
                        cb = slice(cbi * 512, (cbi + 1) * 512)
                        nc.tensor.matmul(
                            pso[:, cb],
                            lhsT=ones_row[0:1, 0:P],
                            rhs=brows["bo"][0:1, cb],
                            start=False,
                            stop=True,
                        )
                    oo = osb.tile([P, d], F32, tag="out_sb")
                    nc.scalar.activation(out=oo, in_=pso, func=AF.Copy)
                    nc.scalar.dma_start(
                        out=dout[mt * P:(mt + 1) * P, :], in_=oo
                    )

    nc.compile()
    return nc


_NC_CACHE = {}


def _get_nc():
    if "nc" not in _NC_CACHE:
        _NC_CACHE["nc"] = build_attention_nc()
    return _NC_CACHE["nc"]


def kernel(k, v, q, attn_mask, Wk, bk, Wv, bv, Wq, bq, Wo, bo, **_ignored):
    import ml_dtypes
    from concourse.bass_utils import run_bass_kernel_spmd

    bf16 = ml_dtypes.bfloat16
    k = np.asarray(k, np.float32)
    v = np.asarray(v, np.float32)
    q = np.asarray(q, np.float32)
    attn_mask = np.asarray(attn_mask)
    # Host-side prep (layout choice, not compute): transpose + bf16-cast the
    # per-core inputs and bf16-cast the shared weights.  The kernel cast
    # everything to bf16 on-chip anyway, so numerics are identical.
    shared = {
        "Wk": np.asarray(Wk, np.float32).astype(bf16),
        "Wv": np.asarray(Wv, np.float32).astype(bf16),
        "Wq": np.asarray(Wq, np.float32).astype(bf16),
        "Wo": np.asarray(Wo, np.float32).astype(bf16),
        "bk": np.asarray(bk, np.float32), "bv": np.asarray(bv, np.float32),
        "bq": np.asarray(bq, np.float32), "bo": np.asarray(bo, np.float32),
    }
    in_maps = []
    for b in range(B):
        mb = np.where(attn_mask[b].T != 0, np.float32(MASK_BIG),
                      np.float32(0.0)).astype(bf16)
        m = {
            "qT": np.ascontiguousarray(q[b].T).astype(bf16),
            "kT": np.ascontiguousarray(k[b].T).astype(bf16),
            "vT": np.ascontiguousarray(v[b].T).astype(bf16),
            "mbT": np.ascontiguousarray(mb),
        }
        m.update(shared)
        in_maps.append(m)

    nc = _get_nc()
    res = run_bass_kernel_spmd(nc, in_maps, core_ids=list(range(B)))
    output = np.stack([np.asarray(res.results[b]["output"], np.float32)
                       for b in range(B)])
    attn_mean = np.stack(
        [np.asarray(res.results[b]["attn_meanT"]).T.astype(np.float32)
         for b in range(B)])
    return output, attn_mean


# revision 23
# speedup vs baseline: 1.2362x; 1.1746x over previous
"""MultiHeadAttention Trainium2 kernel (8-core batch-parallel), v15.

Reference computation (per batch b):
    K = k @ Wk + bk ; V = v @ Wv + bv ; Q = (q @ Wq + bq) * (1/8)
    per head h: scores = Qh @ Kh^T ; scores[mask!=0] = -inf
    attn = softmax(scores, axis=-1)
    context_h = attn @ Vh ; output = concat(context) @ Wo + bo
    attn_mean = sum_h(attn) / 16

Sharding: pure data-parallel over batch (B=8 -> one batch per core).

Per-core design ("transposed softmax", software-pipelined):
  - Inputs cast bf16 + transposed on-chip (PE transpose + DVE copy);
    no DRAM bounce. meanT/ctxT double as phase-0 scratch.
  - Scores PSUM tiles [128, 1024] (2 banks): one exp ACT per (head, kt).
  - Mask on PE: psum += I.T @ (-30000*maskT).
  - Head pairs: two K=64 scores matmuls on PE row groups 0-1 / 2-3.
  - Softmax tail per head (v4 recip path; single ACT table set):
    denoms row -> [128,8] via tiny PE matmuls -> DVE recip -> PE
    transpose -> onehot row-broadcast -> rb16 = 1/(16*denom) bf16.
  - Iteration order: mean_back(pr-1) | scores/mask/exp(pr) | PV(pr) |
    tail(pr).  The DVE mean backlog of pair pr-1 overlaps pair pr's
    dense PE/ACT work; em pool bufs=4 so exp never waits on the mean.
  - attn_mean written transposed bf16, transposed+cast on host.
"""

import numpy as np

import concourse.bass as bass
import concourse.mybir as mybir
import concourse.tile as tile
from concourse import bacc
from concourse.masks import make_identity

F32 = mybir.dt.float32
BF16 = mybir.dt.bfloat16
I32 = mybir.dt.int32
AF = mybir.ActivationFunctionType
OP = mybir.AluOpType

B = 8
S = 1024
D = 1024
H = 16
DH = 64
P = 128

MASK_BIG = -30000.0  # representable in bf16; exp(s + MASK_BIG) == 0 in f32

# tuning knobs
MEAN_GPS_KTS = ()        # kt indices whose mean-accumulate runs on gpsimd
MASK_DVE_KTS = ()        # kts whose mask is DVE (mbT==0)*exp instead of PE


def build_attention_nc(s=S, h=H, debug=False):
    d = D
    nt = d // P          # tiles along d (8)
    st = s // P          # tiles along s (8)
    hpt = P // DH        # heads per 128-partition tile (2)
    npair = h // hpt     # head pairs (8)

    nc = bacc.Bacc("TRN2", target_bir_lowering=False, debug=debug)

    # host-prepped: transposed bf16 inputs ([d, s]) and bf16 weights; the
    # on-chip pipeline cast everything to bf16 anyway, so numerics match.
    dqT = nc.dram_tensor("qT", [d, s], BF16, kind="ExternalInput")
    dkT = nc.dram_tensor("kT", [d, s], BF16, kind="ExternalInput")
    dvT = nc.dram_tensor("vT", [d, s], BF16, kind="ExternalInput")
    dmbT = nc.dram_tensor("mbT", [s, s], BF16, kind="ExternalInput")
    dWq = nc.dram_tensor("Wq", [d, d], BF16, kind="ExternalInput")
    dWk = nc.dram_tensor("Wk", [d, d], BF16, kind="ExternalInput")
    dWv = nc.dram_tensor("Wv", [d, d], BF16, kind="ExternalInput")
    dWo = nc.dram_tensor("Wo", [d, d], BF16, kind="ExternalInput")
    dbq = nc.dram_tensor("bq", [d], F32, kind="ExternalInput")
    dbk = nc.dram_tensor("bk", [d], F32, kind="ExternalInput")
    dbv = nc.dram_tensor("bv", [d], F32, kind="ExternalInput")
    dbo = nc.dram_tensor("bo", [d], F32, kind="ExternalInput")
    dout = nc.dram_tensor("output", [s, d], F32, kind="ExternalOutput")
    # attn_mean, stored transposed ([k, q]); host transposes for free.
    dmeanT = nc.dram_tensor("attn_meanT", [s, s], BF16, kind="ExternalOutput")

    with tile.TileContext(nc) as tc:
        with (
            tc.tile_pool(name="persist", bufs=1) as persist,
            tc.tile_pool(name="consts", bufs=1) as consts,
            tc.tile_pool(name="ps", bufs=2, space="PSUM") as ps,
        ):
            # ---------- constants ----------
            identB = consts.tile([P, P], BF16)
            make_identity(nc, identB)
            ident_f = consts.tile([P, P], F32)
            make_identity(nc, ident_f)
            ones_row = consts.tile([1, s], BF16)
            nc.vector.memset(ones_row, 1.0)
            ones_f32 = consts.tile([1, 1], F32)
            nc.vector.memset(ones_f32, 1.0)
            # onehot[i, j, c] = (i == j), bf16: stationary for row-broadcasts
            onehot = consts.tile([st, st, P], BF16)
            nc.gpsimd.memset(onehot, 0.0)
            nc.gpsimd.affine_select(
                out=onehot, in_=onehot, compare_op=OP.not_equal, fill=1.0,
                base=0, pattern=[[-1, st], [0, P]], channel_multiplier=1,
            )

            # persistent big tensors
            QT = persist.tile([P, nt, s], BF16)
            KT = persist.tile([P, nt, s], BF16)
            Vx = persist.tile([P, st, h, DH + 1], BF16)
            mbT = persist.tile([P, st, s], BF16)   # (-30000 * mask)^T
            ctxT = persist.tile([P, nt, s], BF16)
            meanT = persist.tile([P, st, s], BF16)
            wo = persist.tile([P, nt, d], BF16)

            # per-partition bias columns for Q/K (ScalarE bias path)
            bq8 = consts.tile([P, nt], F32)
            bk_c = consts.tile([P, nt], F32)
            brows = {}

            nc.vector.memset(Vx[:, :, :, DH:DH + 1], 1.0)

            # ---------- phase 0: load (pre-transposed bf16), project ----------
            with (
                tc.tile_pool(name="stage", bufs=1) as stage,
                tc.tile_pool(name="wpool", bufs=2) as wpool,
            ):
                # biases (sync queue; small)
                bqf = stage.tile([P, nt], F32, tag="bias_c", bufs=2)
                nc.sync.dma_start(out=bqf, in_=dbq.rearrange("(i p) -> p i", p=P))
                nc.vector.tensor_scalar(
                    out=bq8, in0=bqf, scalar1=1.0 / 8.0, scalar2=None,
                    op0=OP.mult,
                )
                bkf = stage.tile([P, nt], F32, tag="bias_c", bufs=2)
                nc.sync.dma_start(out=bkf, in_=dbk.rearrange("(i p) -> p i", p=P))
                nc.vector.tensor_copy(out=bk_c, in_=bkf)
                for nm, dt_ in (("bv", dbv), ("bo", dbo)):
                    rf = stage.tile([1, d], F32, tag="brow_st", bufs=1)
                    nc.sync.dma_start(out=rf, in_=dt_[None, :])
                    rb_ = consts.tile([1, d], BF16, tag=f"{nm}b")
                    nc.vector.tensor_copy(out=rb_, in_=rf)
                    brows[nm] = rb_

                def proj_T(wsb, x_T, outbuf, bias_col, scale):
                    """outbuf[dout, s] = ((x @ W) * scale + bias_col)."""
                    for mt in range(nt):
                        psj = ps.tile([P, s], F32, tag="sc", name=f"pj{mt}")
                        for kt in range(nt):
                            for cbi in range(2):
                                cb = slice(cbi * 512, (cbi + 1) * 512)
                                nc.tensor.matmul(
                                    psj[:, cb],
                                    lhsT=wsb[:, kt, mt * P:(mt + 1) * P],
                                    rhs=x_T[:, kt, cb],
                                    start=(kt == 0),
                                    stop=(kt == nt - 1),
                                )
                        nc.scalar.activation(
                            out=outbuf[:, mt, :], in_=psj,
                            func=AF.Identity, scale=scale,
                            bias=bias_col[:, mt:mt + 1],
                        )

                # Direct single-DMA loads. meanT/ctxT/wo double as scratch
                # for qT/kT/vT (their first real writes come later; the
                # region tracker orders the WAR dependencies).
                wq = wpool.tile([P, nt, d], BF16, tag="w", name="wq")
                qT_in = meanT
                wk = wpool.tile([P, nt, d], BF16, tag="w", name="wk")
                kT_in = ctxT
                # chunked loads so the first proj matmuls start early
                for c2 in range(4):
                    rows = slice(c2 * 2 * P, (c2 + 1) * 2 * P)
                    nc.scalar.dma_start(
                        out=wq[:, 2 * c2:2 * c2 + 2, :],
                        in_=dWq[rows, :].rearrange("(c p) d -> p c d", p=P))
                    nc.sync.dma_start(
                        out=qT_in[:, 2 * c2:2 * c2 + 2, :],
                        in_=dqT[rows, :].rearrange("(c p) x -> p c x", p=P))
                for c2 in range(4):
                    rows = slice(c2 * 2 * P, (c2 + 1) * 2 * P)
                    nc.scalar.dma_start(
                        out=wk[:, 2 * c2:2 * c2 + 2, :],
                        in_=dWk[rows, :].rearrange("(c p) d -> p c d", p=P))
                    nc.sync.dma_start(
                        out=kT_in[:, 2 * c2:2 * c2 + 2, :],
                        in_=dkT[rows, :].rearrange("(c p) x -> p c x", p=P))
                for c2 in range(4):
                    rows = slice(c2 * 2 * P, (c2 + 1) * 2 * P)
                    nc.sync.dma_start(
                        out=mbT[:, 2 * c2:2 * c2 + 2, :],
                        in_=dmbT[rows, :].rearrange("(c p) x -> p c x", p=P))
                proj_T(wq, qT_in, QT, bq8, 1.0 / 8.0)
                wv = wpool.tile([P, nt, d], BF16, tag="w", name="wv")
                vT_in = wo
                for c2 in range(4):
                    rows = slice(c2 * 2 * P, (c2 + 1) * 2 * P)
                    nc.scalar.dma_start(
                        out=wv[:, 2 * c2:2 * c2 + 2, :],
                        in_=dWv[rows, :].rearrange("(c p) d -> p c d", p=P))
                    nc.scalar.dma_start(
                        out=vT_in[:, 2 * c2:2 * c2 + 2, :],
                        in_=dvT[rows, :].rearrange("(c p) x -> p c x", p=P))
                proj_T(wk, kT_in, KT, bk_c, 1.0)

                # V: natural layout, scaled by 16, bias via ones-row matmul
                for mt in range(st):
                    psv = ps.tile([P, s], F32, tag="sc", name=f"pv_ps{mt}")
                    for kt in range(nt):
                        for cbi in range(2):
                            cb = slice(cbi * 512, (cbi + 1) * 512)
                            nc.tensor.matmul(
                                psv[:, cb],
                                lhsT=vT_in[:, kt, mt * P:(mt + 1) * P],
                                rhs=wv[:, kt, cb],
                                start=(kt == 0),
                                stop=False,
                            )
                    for cbi in range(2):
                        cb = slice(cbi * 512, (cbi + 1) * 512)
                        nc.tensor.matmul(
                            psv[:, cb],
                            lhsT=ones_row[0:1, 0:P],
                            rhs=brows["bv"][0:1, cb],
                            start=False,
                            stop=True,
                        )
                    nc.scalar.activation(
                        out=Vx[:, mt, :, 0:DH],
                        in_=psv.rearrange("p (hh dh) -> p hh dh", dh=DH),
                        func=AF.Copy, scale=16.0,
                    )

            # ---------- head loop (software-pipelined pairs) ----------
            with (
                tc.tile_pool(name="emp", bufs=4) as emp,
                tc.tile_pool(name="rbp", bufs=3) as rbp,
                tc.tile_pool(name="hwork", bufs=2) as hwork,
            ):
                ems = {}
                ppvs = {}
                rbs = {}

                def dense_kt(pr, kt):
                    """scores + mask + exp for one kt of pair pr."""
                    he, ho_ = 2 * pr, 2 * pr + 1
                    dve_mask = kt in MASK_DVE_KTS
                    pss = {}
                    for hh in (he, ho_):
                        pss[hh] = ps.tile([P, s], F32, tag="sc",
                                          name=f"sc{hh}_{kt}")
                    # adjacent K=64 row-group matmuls (concurrent pair)
                    for cbi in range(2):
                        cb = slice(cbi * 512, (cbi + 1) * 512)
                        for hh in (he, ho_):
                            lo = (hh % hpt) * DH
                            nc.tensor.matmul(
                                pss[hh][:, cb],
                                lhsT=KT[lo:lo + DH, pr,
                                        kt * P:(kt + 1) * P],
                                rhs=QT[lo:lo + DH, pr, cb],
                                start=True,
                                stop=dve_mask,
                                tile_position=(lo, 0),
                            )
                    if not dve_mask:
                        for cbi in range(2):
                            cb = slice(cbi * 512, (cbi + 1) * 512)
                            for hh in (he, ho_):
                                nc.tensor.matmul(
                                    pss[hh][:, cb],
                                    lhsT=identB,
                                    rhs=mbT[:, kt, cb],
                                    start=False,
                                    stop=True,
                                )
                    for hh in (he, ho_):
                        et = ems[hh][:, kt, :]
                        nc.scalar.activation(
                            out=et, in_=pss[hh], func=AF.Exp
                        )
                        if dve_mask:
                            nc.vector.scalar_tensor_tensor(
                                out=et, in0=mbT[:, kt, :],
                                scalar=0.0, in1=et,
                                op0=OP.is_equal, op1=OP.mult,
                            )

                def pv_kt(hh, kt):
                    """PV accumulation matmuls for head hh, one kt."""
                    for cbi in range(2):
                        cb = slice(cbi * 512, (cbi + 1) * 512)
                        nc.tensor.matmul(
                            ppvs[hh][:, cb],
                            lhsT=Vx[:, kt, hh, :],
                            rhs=ems[hh][:, kt, cb],
                            start=(kt == 0),
                            stop=(kt == st - 1),
                        )

                rTs = {}

                def tail_start(hh):
                    """denom row -> rT bf16 [8, 128]; no PE work."""
                    ppv = ppvs[hh]
                    srow = hwork.tile([1, s], F32, tag="srow")
                    nc.scalar.activation(
                        out=srow, in_=ppv[DH:DH + 1, :], func=AF.Copy,
                        scale=16.0,
                    )
                    # reshape [1, 1024] -> [8, 128] with one tiny DMA
                    # (source stays a true single-partition view; the
                    # OUT AP drives the partition scatter), then
                    # reciprocal straight to bf16
                    rpre = hwork.tile([st, P], F32, tag="rpre")
                    nc.sync.dma_start(
                        out=rpre,
                        in_=srow.rearrange("o (j p) -> o j p", p=P),
                    )
                    rT = hwork.tile([st, P], BF16, tag="rT")
                    with nc.allow_low_precision(
                            reason="rb is consumed in bf16 anyway"):
                        nc.vector.reciprocal(out=rT, in_=rpre)
                    rTs[hh] = rT

                def tail_finish(pr, hh):
                    """onehot broadcast + rb + ctx (frees ppv slots)."""
                    lo = (hh % hpt) * DH
                    ppv = ppvs[hh]
                    rT = rTs[hh]
                    prb = ps.tile([P, s], F32, tag="sc", name="prb")
                    for qbi in range(2):
                        for j2 in range(4):
                            j = qbi * 4 + j2
                            nc.tensor.matmul(
                                prb[:, qbi * 512 + j2 * P:
                                    qbi * 512 + (j2 + 1) * P],
                                lhsT=onehot[:, j, :],
                                rhs=rT,
                                start=True, stop=True,
                            )
                    rb = rbp.tile([P, s], BF16, tag="rb")
                    nc.scalar.activation(out=rb, in_=prb, func=AF.Copy)
                    rbs[hh] = rb
                    nc.vector.tensor_tensor(
                        out=ctxT[lo:lo + DH, pr, :], in0=ppv[0:DH, :],
                        in1=rb[0:DH, :], op=OP.mult,
                    )

                def mean_back(pr, emit_dma=False):
                    for hh in (2 * pr, 2 * pr + 1):
                        em = ems[hh]
                        rb = rbs[hh]
                        for kt in range(st):
                            eng = (nc.gpsimd if kt in MEAN_GPS_KTS
                                   else nc.vector)
                            if hh == 0:
                                eng.tensor_tensor(
                                    out=meanT[:, kt, :], in0=em[:, kt, :],
                                    in1=rb, op=OP.mult,
                                )
                            else:
                                eng.tensor_tensor(
                                    out=em[:, kt, :], in0=em[:, kt, :],
                                    in1=rb, op=OP.mult,
                                )
                                eng.tensor_tensor(
                                    out=meanT[:, kt, :], in0=em[:, kt, :],
                                    in1=meanT[:, kt, :], op=OP.add,
                                )
                            if emit_dma and hh == 2 * pr + 1:
                                nc.sync.dma_start(
                                    out=dmeanT[kt * P:(kt + 1) * P, :],
                                    in_=meanT[:, kt, :],
                                )

                # Pipeline: PV matmuls trail the exps by 2-3 kts (never
                # FIFO-waiting on a fresh exp), the previous pair's ho-tail
                # and mean work are emitted inside this pair's dense phase,
                # and tails stagger so the PE never has a low-duty window.
                for pr in range(npair):
                    he, ho_ = 2 * pr, 2 * pr + 1
                    for hh in (he, ho_):
                        ems[hh] = emp.tile([P, st, s], BF16, tag="em",
                                           name=f"em{hh}")
                        ppvs[hh] = ps.tile([DH + 1, s], F32, tag="pv",
                                           name=f"pv{hh}")
                    if pr >= 1:
                        # prev pair's ho chain starts now (no PE ops);
                        # both finishes land under dense kt0/kt1 matmuls
                        tail_start(2 * pr - 1)
                    dense_kt(pr, 0)
                    if pr >= 1:
                        tail_finish(pr - 1, 2 * pr - 2)
                    dense_kt(pr, 1)
                    if pr >= 1:
                        tail_finish(pr - 1, 2 * pr - 1)
                        mean_back(pr - 1)
                    for kt in range(2, st):
                        dense_kt(pr, kt)
                        pv_kt(he, kt - 2)
                        if kt >= 3:
                            pv_kt(ho_, kt - 3)
                    pv_kt(he, st - 2)
                    pv_kt(he, st - 1)
                    tail_start(he)
                    pv_kt(ho_, st - 3)
                    pv_kt(ho_, st - 2)
                    pv_kt(ho_, st - 1)
                    if pr == 1:
                        # Wo load (bf16, single DMA) while the PE is busy
                        nc.scalar.dma_start(
                            out=wo,
                            in_=dWo.rearrange("(c p) d -> p c d", p=P))
                # drain: the epilogue's pre-accumulation (emitted next, in
                # the same ps pool) covers the last pair's tail chains
                tail_finish(npair - 1, 2 * npair - 2)
                tail_start(2 * npair - 1)
                tail_finish(npair - 1, 2 * npair - 1)
                mean_back(npair - 1, emit_dma=True)

            # ---------- epilogue: out = ctxT^T @ Wo + bo ----------
            # mt 0-3 pre-accumulate kt 0-6 (which only needs pairs 0-6's
            # ctx) so the PE stays busy while pair 7's tail + mean drain.
            with tc.tile_pool(name="osb", bufs=3) as osb:
                psos = {}
                for mt in range(4):
                    psos[mt] = ps.tile([P, s], F32,
                                       tag="sc" if mt < 2 else "pv",
                                       name=f"pso{mt}")
                    for kt in range(nt - 1):
                        for cbi in range(2):
                            cb = slice(cbi * 512, (cbi + 1) * 512)
                            nc.tensor.matmul(
                                psos[mt][:, cb],
                                lhsT=ctxT[:, kt, mt * P:(mt + 1) * P],
                                rhs=wo[:, kt, cb],
                                start=(kt == 0),
                                stop=False,
                            )
                for mt in range(st):
                    if mt < 4:
                        pso = psos[mt]
                        kts = [nt - 1]
                    else:
                        pso = ps.tile([P, s], F32,
                                      tag="sc" if mt % 2 == 0 else "pv",
                                      name=f"pso{mt}")
                        kts = list(range(nt))
                    for kt in kts:
                        for cbi in range(2):
                            cb = slice(cbi * 512, (cbi + 1) * 512)
                            nc.tensor.matmul(
                                pso[:, cb],
                                lhsT=ctxT[:, kt, mt * P:(mt + 1) * P],
                                rhs=wo[:, kt, cb],
                                start=(kt == 0 and mt >= 4),
                                stop=False,
                            )
                    for cbi in range(2):
                        cb = slice(cbi * 512, (cbi + 1) * 512)
                        nc.tensor.matmul(
                            pso[:, cb],
                            lhsT=ones_row[0:1, 0:P],
                            rhs=brows["bo"][0:1, cb],
                            start=False,
                            stop=True,
                        )
                    oo = osb.tile([P, d], F32, tag="out_sb")
                    nc.scalar.activation(out=oo, in_=pso, func=AF.Copy)
                    nc.scalar.dma_start(
                        out=dout[mt * P:(mt + 1) * P, :], in_=oo
                    )

    nc.compile()
    return nc


_NC_CACHE = {}


def _get_nc():
    if "nc" not in _NC_CACHE:
        _NC_CACHE["nc"] = build_attention_nc()
    return _NC_CACHE["nc"]


def kernel(k, v, q, attn_mask, Wk, bk, Wv, bv, Wq, bq, Wo, bo, **_ignored):
    import ml_dtypes
    from concourse.bass_utils import run_bass_kernel_spmd

    bf16 = ml_dtypes.bfloat16
    k = np.asarray(k, np.float32)
    v = np.asarray(v, np.float32)
    q = np.asarray(q, np.float32)
    attn_mask = np.asarray(attn_mask)
    # Host-side prep (layout choice, not compute): transpose + bf16-cast the
    # per-core inputs and bf16-cast the shared weights.  The kernel cast
    # everything to bf16 on-chip anyway, so numerics are identical.
    shared = {
        "Wk": np.asarray(Wk, np.float32).astype(bf16),
        "Wv": np.asarray(Wv, np.float32).astype(bf16),
        "Wq": np.asarray(Wq, np.float32).astype(bf16),
        "Wo": np.asarray(Wo, np.float32).astype(bf16),
        "bk": np.asarray(bk, np.float32), "bv": np.asarray(bv, np.float32),
        "bq": np.asarray(bq, np.float32), "bo": np.asarray(bo, np.float32),
    }
    in_maps = []
    for b in range(B):
        mb = np.where(attn_mask[b].T != 0, np.float32(MASK_BIG),
                      np.float32(0.0)).astype(bf16)
        m = {
            "qT": np.ascontiguousarray(q[b].T).astype(bf16),
            "kT": np.ascontiguousarray(k[b].T).astype(bf16),
            "vT": np.ascontiguousarray(v[b].T).astype(bf16),
            "mbT": np.ascontiguousarray(mb),
        }
        m.update(shared)
        in_maps.append(m)

    nc = _get_nc()
    res = run_bass_kernel_spmd(nc, in_maps, core_ids=list(range(B)))
    output = np.stack([np.asarray(res.results[b]["output"], np.float32)
                       for b in range(B)])
    attn_mean = np.stack(
        [np.asarray(res.results[b]["attn_meanT"]).T.astype(np.float32)
         for b in range(B)])
    return output, attn_mean
